# revision 20
# baseline (speedup 1.0000x reference)
"""Multi-head attention (B=8, N=1024, C=768, H=12) on 8 Trainium2 NeuronCores.

Sharding: data-parallel, one batch element per core. Each core computes the
full attention block for its batch: QKV projection, per-head softmax(QK^T/8)V,
and the output projection, entirely on-chip (SBUF/PSUM).

Layout strategy (chosen so no on-device transposes are needed):
  - host passes x^T and the weights pre-tiled to [128, KT, cols]
    (partition-major k-tiling) and regrouped in consumption order, so each
    tensor loads with a handful of large contiguous DMAs. All bf16.
  - Q, K are produced transposed ([d, n], head-dim on partitions) by the QKV
    matmul; V is produced in natural [n, d] layout by swapping lhsT/rhs.
  - scores are computed transposed (S^T[m, n] = K Q^T) so that exp(S^T) can be
    consumed directly as the moving operand of the P@V matmul.
  - V tiles carry an appended ones-column, so the P@V matmul's 65th output row
    is the softmax denominator (row-sum of exp scores) for free.
  - normalization multiplies by a reciprocal row broadcast across partitions
    via a DRAM-bounced DMA (SBUF APs cannot partition-broadcast).

Dtypes: everything the PE touches is bf16 (halves DMA, enables fast weight
load, and keeps the PE out of FP32-HIGH mode, which would block background
weight loads); accumulation and the softmax normalization stay f32.

Scheduling: emission order is the Tile scheduler's priority. Head pairs 0/1
and all V tiles are produced up front; attention for pair t overlaps the
remaining Q/K projection (pair t+2's two m-tiles are emitted at the pair's
two j-boundaries, halving the exp-stream bubble a single burst would cause).
The output projection is emitted last; its first four token tiles only need
attn-out columns 0:512, so they are emitted right after the final pair's
j=0 normalization and overlap its j=1 attention.
"""

import sys

import numpy as np

if "/opt/trn_rl_repo" not in sys.path:
    sys.path.insert(0, "/opt/trn_rl_repo")

B = 8
N = 1024
C = 768
H = 12
D = 64
SCALE = D ** -0.5
KT = C // 128           # 6 contraction tiles over channels
NT = N // 128           # 8 token tiles
PAIRS = H // 2          # 6 head pairs

# m-tile consumption order for Q/K projection: qkt[m] holds heads 2m/2m+1
# (m 0..5 = Q) or K for pair m-6 (m 6..11). Pairs 0/1 run first, then pair
# t+2 is produced while pair t's attention runs. wqkA holds the m-tiles for
# pairs 0/1 as two contiguous 256-col blocks (m0|m6, m1|m7); wqkB holds
# pairs 2..5 as four contiguous blocks (m2|m8, ..., m5|m11).
_CACHE = {}


def build_program(fast=True):
    import concourse.bacc as bacc
    import concourse.mybir as mybir
    import concourse.tile as tile

    f32 = mybir.dt.float32
    bf16 = mybir.dt.bfloat16
    Exp = mybir.ActivationFunctionType.Exp
    fm = bf16 if fast else f32

    nc = bacc.Bacc("TRN2", target_bir_lowering=False, debug=False)

    xT_d = nc.dram_tensor("xT", [128, KT, N], fm, kind="ExternalInput")
    wqkA_d = nc.dram_tensor("wqkA", [128, 2, KT, 256], fm,
                            kind="ExternalInput")
    wqkB_d = nc.dram_tensor("wqkB", [128, 4, KT, 256], fm,
                            kind="ExternalInput")
    wv_d = nc.dram_tensor("wvT", [128, KT, C], fm, kind="ExternalInput")
    wprojT_d = nc.dram_tensor("wprojT", [128, KT, C], fm,
                              kind="ExternalInput")
    bias_d = nc.dram_tensor("bias_rep", [128, C], f32, kind="ExternalInput")
    y_d = nc.dram_tensor("y", [N, C], f32, kind="ExternalOutput")

    mm = nc.tensor.matmul

    with tile.TileContext(nc) as tc:
        # qkt/aot share one 12-slot tag: each aot[t] lands in the slot of a
        # Q^T/K^T tile that died right before it (pair t's score matmuls).
        with tc.tile_pool(name="pers", bufs=1) as pers, \
             tc.tile_pool(name="qa", bufs=13) as qa, \
             tc.tile_pool(name="cyc", bufs=2) as pB, \
             tc.tile_pool(name="dramb", bufs=2, space="DRAM") as pDr, \
             tc.tile_pool(name="ps_s", bufs=3, space="PSUM") as psS, \
             tc.tile_pool(name="ps_y", bufs=2, space="PSUM") as psY:
            # Q^T,K^T tiles [d, n]: tile m holds heads 2m (parts 0:64) and
            # 2m+1 (parts 64:128); m 0..5 = Q, 6..11 = K.
            qkt = [qa.tile([128, N], fm, name=f"qkt{m}", tag="qa")
                   for m in range(2 * KT)]
            # V tiles [n-tile, pair, 130]: per pair block [V_h0 |1| V_h1 |1];
            # ones cols at 64 and 129 feed the denominator row of P@V.
            vbuf = [pers.tile([128, PAIRS, 130], fm, name=f"vbuf{i}",
                              tag=f"vbuf{i}")
                    for i in range(NT)]
            # w_proj/bias load right after the QKV weights on the ACT ring:
            # emitting their DMAs late would queue them behind all 96 exp
            # activations on the ACT engine and stall the output projection.
            wp = pers.tile([128, KT, C], fm, name="wp", tag="wp")
            bias_t = pers.tile([128, C], f32, name="bias_t", tag="bias_t")

            with tc.tile_pool(name="phA", bufs=1) as pA:
                xt = pA.tile([128, KT, N], fm, name="xt", tag="xt")
                wqkA = pA.tile([128, 2, KT, 256], fm, name="wqkA", tag="wqkA")
                wqkB = pA.tile([128, 4, KT, 256], fm, name="wqkB", tag="wqkB")
                wv = pA.tile([128, KT, C], fm, name="wv", tag="wv")
                # x^T per-k on the SP HWDGE ring (contiguous 2KB/partition
                # slices, first k lands ~1.5us after issue); weights on the
                # ACT ring in consumption order, pair-blocks contiguous.
                for k in range(KT):
                    nc.sync.dma_start(xt[:, k, :], xT_d[:, k, :])
                nc.scalar.dma_start(wqkA[:, 0], wqkA_d[:, 0])
                nc.scalar.dma_start(wqkA[:, 1], wqkA_d[:, 1])
                nc.scalar.dma_start(wv[:], wv_d[:])
                nc.scalar.dma_start(wqkB[:], wqkB_d[:])
                nc.scalar.dma_start(wp[:], wprojT_d[:])
                nc.scalar.dma_start(bias_t[:], bias_d[:])
                for i in range(NT):
                    ones_ap = vbuf[i].rearrange("p a (t c) -> p a t c",
                                                c=65)[:, :, :, 64]
                    nc.vector.memset(ones_ap, 1.0)

                # ---- QKV projection, single-bank accumulation groups ----
                # pair p's Q and K m-tiles live in block order: Q at
                # half=0, K at half=1 of the 256-col pair block.
                def emit_qk(p, half):
                    m = p + PAIRS * half
                    wt = wqkA if p < 2 else wqkB
                    blk, c0 = (p, 128 * half) if p < 2 else (p - 2, 128 * half)
                    for j in range(2):
                        ps = psS.tile([128, 512], f32, name="qk_ps", tag="ps")
                        for k in range(KT):
                            mm(ps[:], wt[:, blk, k, c0:c0 + 128],
                               xt[:, k, 512 * j:512 * (j + 1)],
                               start=(k == 0), stop=(k == KT - 1))
                        nc.vector.tensor_copy(qkt[m][:, 512 * j:512 * (j + 1)],
                                              ps[:])

                def emit_v(i):
                    for c0, w in ((0, 512), (512, 256)):
                        ps = psY.tile([128, 512], f32, name="v_ps", tag="py")
                        for k in range(KT):
                            mm(ps[:, 0:w], xt[:, k, 128 * i:128 * (i + 1)],
                               wv[:, k, c0:c0 + w],
                               start=(k == 0), stop=(k == KT - 1))
                        # scatter heads: even -> cols 0:64, odd -> cols 65:129
                        # within each 130-wide pair block
                        v_view = ps[:, 0:w].rearrange("p (a t c) -> p a t c",
                                                      t=2, c=64)
                        pa0 = c0 // 128
                        npair = w // 128
                        nc.vector.tensor_copy(
                            vbuf[i][:, pa0:pa0 + npair, 0:64], v_view[:, :, 0, :])
                        nc.vector.tensor_copy(
                            vbuf[i][:, pa0:pa0 + npair, 65:129], v_view[:, :, 1, :])

                # head pairs 0/1 first so attention starts while the rest
                # of the QKV projection still runs; remaining Q/K tiles are
                # emitted at the j-boundaries inside the attention loop
                # (emission order drives scheduler priority).
                for p in (0, 1):
                    emit_qk(p, 0)
                    emit_qk(p, 1)
                for i in range(NT):
                    emit_v(i)

                # ---- attention, j-outer so P@V psum is one bank per head ----
                for t in range(PAIRS):
                    qt, kt = qkt[t], qkt[PAIRS + t]
                    aot = qa.tile([128, N], fm, name=f"aot{t}", tag="qa")
                    if t == 0:
                        aot_all = []
                    aot_all.append(aot)
                    for j in range(2):
                        # pair t+2's Q (j=0) / K (j=1) projection: one
                        # m-tile per j-boundary halves the burst that would
                        # otherwise bubble the exp stream at the pair top.
                        if t + 2 < PAIRS:
                            emit_qk(t + 2, j)
                        pv_ps = [psY.tile([65, 512], f32, name=f"pv{h}", tag="py")
                                 for h in range(2)]
                        for i in range(NT):
                            stexp = pB.tile([128, 2, 512], fm, name="stexp",
                                            tag="stexp", bufs=6)
                            s_ps = psS.tile([128, 1024], f32, name="s_ps",
                                            tag="ps")
                            for h in range(2):
                                # S^T[m, n] = sum_d K^T[d, m] Q^T[d, n]; h0/h1
                                # use distinct PE row groups (base partition
                                # 0 / 64) and run concurrently.
                                mm(s_ps[:, 512 * h:512 * (h + 1)],
                                   kt[64 * h:64 * (h + 1), 128 * i:128 * (i + 1)],
                                   qt[64 * h:64 * (h + 1), 512 * j:512 * (j + 1)],
                                   start=True, stop=True)
                            # exp(S^T / 8) for both heads, PSUM -> SBUF bf16
                            nc.scalar.activation(
                                stexp[:, :, :],
                                s_ps[:].rearrange("p (h n) -> p h n", h=2),
                                Exp, scale=SCALE)
                            for h in range(2):
                                # rows 0:64 = (P~ @ V)^T, row 64 = denominator
                                mm(pv_ps[h][:],
                                   vbuf[i][:, t, 65 * h:65 * (h + 1)],
                                   stexp[:, h, :],
                                   start=(i == 0), stop=(i == NT - 1))

                        # normalization, phase-ordered so no DVE op ever
                        # head-of-line-blocks the next pair's PSUM release:
                        # copies free the P@V banks immediately; the
                        # DMA-latency-bound multiplies run last.
                        stages = []
                        for h in range(2):
                            stage = pB.tile([65, 512], f32, name="stage",
                                            tag="stage")
                            nc.vector.tensor_copy(stage[:], pv_ps[h][:])
                            stages.append(stage)
                        # [1, 512] DVE reciprocal is FD-bound (~3us); DMA
                        # both heads' denominator rows into one [128, 8]
                        # tile where the same op is ~130ns.
                        den_t = pB.tile([128, 2, 4], f32, name="den_t",
                                        tag="den_t")
                        for h in range(2):
                            nc.sync.dma_start(den_t[:, h, :],
                                              stages[h][64:65, :])
                        nc.vector.reciprocal(den_t[:], den_t[:])
                        dr2 = pDr.tile([2, 512], f32, name="dr2", tag="dr2")
                        nc.sync.dma_start(
                            dr2[:].rearrange("t (a b) -> a t b", b=4),
                            den_t[:])
                        rbs = []
                        for h in range(2):
                            # partition-broadcast of the reciprocal row: SBUF
                            # APs can't have zero partition step, so broadcast
                            # from DRAM.
                            rb = pB.tile([64, 512], f32, name="rb", tag="rb")
                            nc.sync.dma_start(
                                rb[:], dr2[h:h + 1, :].to_broadcast((64, 512)))
                            rbs.append(rb)
                        for h in range(2):
                            if h == 0:
                                nc.vector.tensor_mul(
                                    aot[0:64, 512 * j:512 * (j + 1)],
                                    stages[0][0:64, :], rbs[0][:])
                            else:
                                tmp = pB.tile([64, 512], fm, name="tmp1",
                                              tag="tmp1")
                                nc.vector.tensor_mul(tmp[:], stages[1][0:64, :],
                                                     rbs[1][:])
                                # DVE lanes cannot shift partitions; DMA moves
                                # the odd head into partitions 64:128.
                                nc.sync.dma_start(
                                    aot[64:128, 512 * j:512 * (j + 1)], tmp[:])

                        # output projection y = attn_out^T.T @ w_proj^T + b.
                        # Token tiles 0..3 only read attn-out columns 0:512,
                        # so they can overlap the final pair's j=1 attention.
                        def emit_proj(i):
                            yt = pB.tile([128, C], f32, name="yt", tag="yt")
                            for c0 in (0, 384):
                                if (2 * i + c0 // 384) % 2 == 0:
                                    pp = psS.tile([128, 384], f32, name="pp",
                                                  tag="ps")
                                else:
                                    pp = psY.tile([128, 384], f32, name="pp",
                                                  tag="py")
                                for k in range(KT):
                                    mm(pp[:, 0:384],
                                       aot_all[k][:, 128 * i:128 * (i + 1)],
                                       wp[:, k, c0:c0 + 384],
                                       start=(k == 0), stop=(k == KT - 1))
                                nc.vector.tensor_add(
                                    yt[:, c0:c0 + 384], pp[:, 0:384],
                                    bias_t[:, c0:c0 + 384])
                            nc.sync.dma_start(y_d[128 * i:128 * (i + 1), :],
                                              yt[:])

                        if t == PAIRS - 1 and j == 0:
                            for i in range(4):
                                emit_proj(i)

                for i in range(4, NT):
                    emit_proj(i)

    nc.compile()
    return nc


def to_bf16(a):
    import ml_dtypes
    return np.ascontiguousarray(a, dtype=np.float32).astype(ml_dtypes.bfloat16)


def tile_k(a):
    """[C, cols] -> [128, KT, cols]: partition-major k-tiling so one DMA
    loads all KT contraction tiles."""
    cols = a.shape[1]
    return np.ascontiguousarray(
        a.reshape(KT, 128, cols).transpose(1, 0, 2))


def make_in_maps(x, w_qkv, w_proj, b_proj):
    wqkvT = np.asarray(w_qkv, dtype=np.float32).T      # [C, 3C]
    # qk_cols[c, m, :]: m-tile columns of W_{q,k}^T. Pair block for pair p =
    # (Q tile m=p | K tile m=p+6), 256 cols contiguous.
    qk_cols = wqkvT[:, 0:2 * C].reshape(C, 2 * PAIRS, 128)
    blocks = []
    for p in range(PAIRS):
        blocks.append(np.concatenate(
            [qk_cols[:, p, :], qk_cols[:, PAIRS + p, :]], axis=1))  # [C, 256]
    wqkA = tile_k(to_bf16(np.concatenate(blocks[0:2], axis=1)))
    wqkB = tile_k(to_bf16(np.concatenate(blocks[2:6], axis=1)))
    wqkA = np.ascontiguousarray(
        wqkA.reshape(128, KT, 2, 256).transpose(0, 2, 1, 3))
    wqkB = np.ascontiguousarray(
        wqkB.reshape(128, KT, 4, 256).transpose(0, 2, 1, 3))
    wvT = tile_k(to_bf16(wqkvT[:, 2 * C:3 * C]))
    wprojT = tile_k(to_bf16(np.asarray(w_proj, dtype=np.float32).T))
    bias_rep = np.ascontiguousarray(
        np.broadcast_to(np.asarray(b_proj, dtype=np.float32), (128, C)))
    x = np.asarray(x, dtype=np.float32)
    return [
        {
            "xT": tile_k(to_bf16(x[b].T)),
            "wqkA": wqkA,
            "wqkB": wqkB,
            "wvT": wvT,
            "wprojT": wprojT,
            "bias_rep": bias_rep,
        }
        for b in range(B)
    ]


def kernel(x, w_qkv, w_proj, b_proj):
    from concourse.bass_utils import run_bass_kernel_spmd

    if "nc" not in _CACHE:
        _CACHE["nc"] = build_program()
    nc = _CACHE["nc"]

    in_maps = make_in_maps(x, w_qkv, w_proj, b_proj)
    res = run_bass_kernel_spmd(nc, in_maps, core_ids=list(range(B)))
    out = np.stack([res.results[b]["y"] for b in range(B)], axis=0)
    return out.astype(np.float32)


# revision 21
# speedup vs baseline: 1.0965x; 1.0965x over previous
"""Multi-head attention (B=8, N=1024, C=768, H=12) on 8 Trainium2 NeuronCores.

Sharding: data-parallel, one batch element per core. Each core computes the
full attention block for its batch: QKV projection, per-head softmax(QK^T/8)V,
and the output projection, entirely on-chip (SBUF/PSUM).

Layout strategy (chosen so no on-device transposes are needed):
  - host passes x^T and the weights pre-tiled to [128, KT, cols]
    (partition-major k-tiling) and regrouped in consumption order, so each
    tensor loads with a handful of large contiguous DMAs. All bf16.
  - Q, K are produced transposed ([d, n], head-dim on partitions) by the QKV
    matmul; V is produced in natural [n, d] layout by swapping lhsT/rhs.
  - scores are computed transposed (S^T[m, n] = K Q^T) so that exp(S^T) can be
    consumed directly as the moving operand of the P@V matmul.
  - V tiles carry an appended ones-column, so the P@V matmul's 65th output row
    is the softmax denominator (row-sum of exp scores) for free.
  - normalization multiplies by a reciprocal row broadcast across partitions
    via a DRAM-bounced DMA (SBUF APs cannot partition-broadcast).

Dtypes: everything the PE touches is bf16 (halves DMA, enables fast weight
load, and keeps the PE out of FP32-HIGH mode, which would block background
weight loads); accumulation and the softmax normalization stay f32.

Scheduling: emission order is the Tile scheduler's priority. Head pairs 0/1
and all V tiles are produced up front; attention for pair t overlaps the
remaining Q/K projection (pair t+2's two m-tiles are emitted at the pair's
two j-boundaries, halving the exp-stream bubble a single burst would cause).
The output projection is emitted last; its first four token tiles only need
attn-out columns 0:512, so they are emitted right after the final pair's
j=0 normalization and overlap its j=1 attention.
"""

import sys

import numpy as np

if "/opt/trn_rl_repo" not in sys.path:
    sys.path.insert(0, "/opt/trn_rl_repo")

B = 8
N = 1024
C = 768
H = 12
D = 64
SCALE = D ** -0.5
KT = C // 128           # 6 contraction tiles over channels
NT = N // 128           # 8 token tiles
PAIRS = H // 2          # 6 head pairs

# m-tile consumption order for Q/K projection: qkt[m] holds heads 2m/2m+1
# (m 0..5 = Q) or K for pair m-6 (m 6..11). Pairs 0/1 run first, then pair
# t+2 is produced while pair t's attention runs. wqkA holds the m-tiles for
# pairs 0/1 as two contiguous 256-col blocks (m0|m6, m1|m7); wqkB holds
# pairs 2..5 as four contiguous blocks (m2|m8, ..., m5|m11).
_CACHE = {}


def build_program(fast=True):
    import concourse.bacc as bacc
    import concourse.mybir as mybir
    import concourse.tile as tile

    f32 = mybir.dt.float32
    bf16 = mybir.dt.bfloat16
    Exp = mybir.ActivationFunctionType.Exp
    fm = bf16 if fast else f32

    nc = bacc.Bacc("TRN2", target_bir_lowering=False, debug=False)

    xT_d = nc.dram_tensor("xT", [128, KT, N], fm, kind="ExternalInput")
    wqkA_d = nc.dram_tensor("wqkA", [128, 2, KT, 256], fm,
                            kind="ExternalInput")
    wqkB_d = nc.dram_tensor("wqkB", [128, 4, KT, 256], fm,
                            kind="ExternalInput")
    wv_d = nc.dram_tensor("wvT", [128, KT, C], fm, kind="ExternalInput")
    wprojT_d = nc.dram_tensor("wprojT", [128, KT, C], fm,
                              kind="ExternalInput")
    bias_d = nc.dram_tensor("bias_rep", [128, C], f32, kind="ExternalInput")
    y_d = nc.dram_tensor("y", [N, C], f32, kind="ExternalOutput")

    mm = nc.tensor.matmul

    with tile.TileContext(nc) as tc:
        # qkt/aot share one 12-slot tag: each aot[t] lands in the slot of a
        # Q^T/K^T tile that died right before it (pair t's score matmuls).
        with tc.tile_pool(name="pers", bufs=1) as pers, \
             tc.tile_pool(name="qa", bufs=13) as qa, \
             tc.tile_pool(name="cyc", bufs=2) as pB, \
             tc.tile_pool(name="dramb", bufs=2, space="DRAM") as pDr, \
             tc.tile_pool(name="ps_s", bufs=3, space="PSUM") as psS, \
             tc.tile_pool(name="ps_y", bufs=2, space="PSUM") as psY:
            # Q^T,K^T tiles [d, n]: tile m holds heads 2m (parts 0:64) and
            # 2m+1 (parts 64:128); m 0..5 = Q, 6..11 = K.
            qkt = [qa.tile([128, N], fm, name=f"qkt{m}", tag="qa")
                   for m in range(2 * KT)]
            # V tiles [n-tile, pair, 130]: per pair block [V_h0 |1| V_h1 |1];
            # ones cols at 64 and 129 feed the denominator row of P@V.
            vbuf = [pers.tile([128, PAIRS, 130], fm, name=f"vbuf{i}",
                              tag=f"vbuf{i}")
                    for i in range(NT)]
            # w_proj/bias load right after the QKV weights on the ACT ring:
            # emitting their DMAs late would queue them behind all 96 exp
            # activations on the ACT engine and stall the output projection.
            wp = pers.tile([128, KT, C], fm, name="wp", tag="wp")
            bias_t = pers.tile([128, C], f32, name="bias_t", tag="bias_t")

            with tc.tile_pool(name="phA", bufs=1) as pA:
                xt = pA.tile([128, KT, N], fm, name="xt", tag="xt")
                wqkA = pA.tile([128, 2, KT, 256], fm, name="wqkA", tag="wqkA")
                wqkB = pA.tile([128, 4, KT, 256], fm, name="wqkB", tag="wqkB")
                wv = pA.tile([128, KT, C], fm, name="wv", tag="wv")
                # x^T per-k on the SP HWDGE ring (contiguous 2KB/partition
                # slices, first k lands ~1.5us after issue); weights on the
                # ACT ring in consumption order, pair-blocks contiguous.
                for k in range(KT):
                    nc.sync.dma_start(xt[:, k, :], xT_d[:, k, :])
                nc.scalar.dma_start(wqkA[:, 0], wqkA_d[:, 0])
                nc.scalar.dma_start(wqkA[:, 1], wqkA_d[:, 1])
                nc.scalar.dma_start(wv[:], wv_d[:])
                nc.scalar.dma_start(wqkB[:], wqkB_d[:])
                nc.scalar.dma_start(wp[:], wprojT_d[:])
                nc.scalar.dma_start(bias_t[:], bias_d[:])
                for i in range(NT):
                    ones_ap = vbuf[i].rearrange("p a (t c) -> p a t c",
                                                c=65)[:, :, :, 64]
                    nc.vector.memset(ones_ap, 1.0)

                # ---- QKV projection, single-bank accumulation groups ----
                # pair p's Q and K m-tiles live in block order: Q at
                # half=0, K at half=1 of the 256-col pair block.
                def emit_qk(p, half):
                    m = p + PAIRS * half
                    wt = wqkA if p < 2 else wqkB
                    blk, c0 = (p, 128 * half) if p < 2 else (p - 2, 128 * half)
                    for j in range(2):
                        ps = psS.tile([128, 512], f32, name="qk_ps", tag="ps")
                        for k in range(KT):
                            mm(ps[:], wt[:, blk, k, c0:c0 + 128],
                               xt[:, k, 512 * j:512 * (j + 1)],
                               start=(k == 0), stop=(k == KT - 1))
                        nc.vector.tensor_copy(qkt[m][:, 512 * j:512 * (j + 1)],
                                              ps[:])

                def emit_v(i):
                    for c0, w in ((0, 512), (512, 256)):
                        ps = psY.tile([128, 512], f32, name="v_ps", tag="py")
                        for k in range(KT):
                            mm(ps[:, 0:w], xt[:, k, 128 * i:128 * (i + 1)],
                               wv[:, k, c0:c0 + w],
                               start=(k == 0), stop=(k == KT - 1))
                        # scatter heads: even -> cols 0:64, odd -> cols 65:129
                        # within each 130-wide pair block
                        v_view = ps[:, 0:w].rearrange("p (a t c) -> p a t c",
                                                      t=2, c=64)
                        pa0 = c0 // 128
                        npair = w // 128
                        nc.vector.tensor_copy(
                            vbuf[i][:, pa0:pa0 + npair, 0:64], v_view[:, :, 0, :])
                        nc.vector.tensor_copy(
                            vbuf[i][:, pa0:pa0 + npair, 65:129], v_view[:, :, 1, :])

                # head pairs 0/1 first so attention starts while the rest
                # of the QKV projection still runs; remaining Q/K tiles are
                # emitted at the j-boundaries inside the attention loop
                # (emission order drives scheduler priority).
                for p in (0, 1):
                    emit_qk(p, 0)
                    emit_qk(p, 1)
                for i in range(NT):
                    emit_v(i)

                # ---- attention, j-outer so P@V psum is one bank per head ----
                for t in range(PAIRS):
                    qt, kt = qkt[t], qkt[PAIRS + t]
                    aot = qa.tile([128, N], fm, name=f"aot{t}", tag="qa")
                    if t == 0:
                        aot_all = []
                    aot_all.append(aot)
                    if t + 2 < PAIRS:
                        emit_qk(t + 2, 0)
                        emit_qk(t + 2, 1)
                    for j in range(2):
                        pv_ps = [psY.tile([65, 512], f32, name=f"pv{h}", tag="py")
                                 for h in range(2)]
                        for i in range(NT):
                            stexp = pB.tile([128, 2, 512], fm, name="stexp",
                                            tag="stexp", bufs=4)
                            s_ps = psS.tile([128, 1024], f32, name="s_ps",
                                            tag="ps")
                            for h in range(2):
                                # S^T[m, n] = sum_d K^T[d, m] Q^T[d, n]; h0/h1
                                # use distinct PE row groups (base partition
                                # 0 / 64) and run concurrently.
                                mm(s_ps[:, 512 * h:512 * (h + 1)],
                                   kt[64 * h:64 * (h + 1), 128 * i:128 * (i + 1)],
                                   qt[64 * h:64 * (h + 1), 512 * j:512 * (j + 1)],
                                   start=True, stop=True)
                            # exp(S^T / 8) for both heads, PSUM -> SBUF bf16
                            nc.scalar.activation(
                                stexp[:, :, :],
                                s_ps[:].rearrange("p (h n) -> p h n", h=2),
                                Exp, scale=SCALE)
                            for h in range(2):
                                # rows 0:64 = (P~ @ V)^T, row 64 = denominator
                                mm(pv_ps[h][:],
                                   vbuf[i][:, t, 65 * h:65 * (h + 1)],
                                   stexp[:, h, :],
                                   start=(i == 0), stop=(i == NT - 1))

                        # normalization, phase-ordered so no DVE op ever
                        # head-of-line-blocks the next pair's PSUM release:
                        # copies free the P@V banks immediately; the
                        # DMA-latency-bound multiplies run last.
                        stages = []
                        for h in range(2):
                            stage = pB.tile([65, 512], f32, name="stage",
                                            tag="stage")
                            nc.vector.tensor_copy(stage[:], pv_ps[h][:])
                            stages.append(stage)
                        # [1, 512] DVE reciprocal is FD-bound (~3us); DMA
                        # both heads' denominator rows into one [128, 8]
                        # tile where the same op is ~130ns.
                        den_t = pB.tile([128, 2, 4], f32, name="den_t",
                                        tag="den_t")
                        for h in range(2):
                            nc.sync.dma_start(den_t[:, h, :],
                                              stages[h][64:65, :])
                        nc.vector.reciprocal(den_t[:], den_t[:])
                        dr2 = pDr.tile([2, 512], f32, name="dr2", tag="dr2")
                        nc.sync.dma_start(
                            dr2[:].rearrange("t (a b) -> a t b", b=4),
                            den_t[:])
                        rbs = []
                        for h in range(2):
                            # partition-broadcast of the reciprocal row: SBUF
                            # APs can't have zero partition step, so broadcast
                            # from DRAM.
                            rb = pB.tile([64, 512], f32, name="rb", tag="rb")
                            nc.sync.dma_start(
                                rb[:], dr2[h:h + 1, :].to_broadcast((64, 512)))
                            rbs.append(rb)
                        for h in range(2):
                            if h == 0:
                                nc.vector.tensor_mul(
                                    aot[0:64, 512 * j:512 * (j + 1)],
                                    stages[0][0:64, :], rbs[0][:])
                            else:
                                tmp = pB.tile([64, 512], fm, name="tmp1",
                                              tag="tmp1")
                                nc.vector.tensor_mul(tmp[:], stages[1][0:64, :],
                                                     rbs[1][:])
                                # DVE lanes cannot shift partitions; DMA moves
                                # the odd head into partitions 64:128.
                                nc.sync.dma_start(
                                    aot[64:128, 512 * j:512 * (j + 1)], tmp[:])

                        # output projection y = attn_out^T.T @ w_proj^T + b.
                        # Token tiles 0..3 only read attn-out columns 0:512,
                        # so they can overlap the final pair's j=1 attention.
                        def emit_proj(i):
                            yt = pB.tile([128, C], f32, name="yt", tag="yt")
                            for c0 in (0, 384):
                                if (2 * i + c0 // 384) % 2 == 0:
                                    pp = psS.tile([128, 384], f32, name="pp",
                                                  tag="ps")
                                else:
                                    pp = psY.tile([128, 384], f32, name="pp",
                                                  tag="py")
                                for k in range(KT):
                                    mm(pp[:, 0:384],
                                       aot_all[k][:, 128 * i:128 * (i + 1)],
                                       wp[:, k, c0:c0 + 384],
                                       start=(k == 0), stop=(k == KT - 1))
                                nc.vector.tensor_add(
                                    yt[:, c0:c0 + 384], pp[:, 0:384],
                                    bias_t[:, c0:c0 + 384])
                            nc.gpsimd.dma_start(
                                y_d[128 * i:128 * (i + 1), :], yt[:])

                        if t == PAIRS - 1 and j == 0:
                            for i in range(4):
                                emit_proj(i)

                for i in range(4, NT):
                    emit_proj(i)

    nc.compile()
    return nc


def to_bf16(a):
    import ml_dtypes
    return np.ascontiguousarray(a, dtype=np.float32).astype(ml_dtypes.bfloat16)


def tile_k(a):
    """[C, cols] -> [128, KT, cols]: partition-major k-tiling so one DMA
    loads all KT contraction tiles."""
    cols = a.shape[1]
    return np.ascontiguousarray(
        a.reshape(KT, 128, cols).transpose(1, 0, 2))


def make_in_maps(x, w_qkv, w_proj, b_proj):
    wqkvT = np.asarray(w_qkv, dtype=np.float32).T      # [C, 3C]
    # qk_cols[c, m, :]: m-tile columns of W_{q,k}^T. Pair block for pair p =
    # (Q tile m=p | K tile m=p+6), 256 cols contiguous.
    qk_cols = wqkvT[:, 0:2 * C].reshape(C, 2 * PAIRS, 128)
    blocks = []
    for p in range(PAIRS):
        blocks.append(np.concatenate(
            [qk_cols[:, p, :], qk_cols[:, PAIRS + p, :]], axis=1))  # [C, 256]
    wqkA = tile_k(to_bf16(np.concatenate(blocks[0:2], axis=1)))
    wqkB = tile_k(to_bf16(np.concatenate(blocks[2:6], axis=1)))
    wqkA = np.ascontiguousarray(
        wqkA.reshape(128, KT, 2, 256).transpose(0, 2, 1, 3))
    wqkB = np.ascontiguousarray(
        wqkB.reshape(128, KT, 4, 256).transpose(0, 2, 1, 3))
    wvT = tile_k(to_bf16(wqkvT[:, 2 * C:3 * C]))
    wprojT = tile_k(to_bf16(np.asarray(w_proj, dtype=np.float32).T))
    bias_rep = np.ascontiguousarray(
        np.broadcast_to(np.asarray(b_proj, dtype=np.float32), (128, C)))
    x = np.asarray(x, dtype=np.float32)
    return [
        {
            "xT": tile_k(to_bf16(x[b].T)),
            "wqkA": wqkA,
            "wqkB": wqkB,
            "wvT": wvT,
            "wprojT": wprojT,
            "bias_rep": bias_rep,
        }
        for b in range(B)
    ]


def kernel(x, w_qkv, w_proj, b_proj):
    from concourse.bass_utils import run_bass_kernel_spmd

    if "nc" not in _CACHE:
        _CACHE["nc"] = build_program()
    nc = _CACHE["nc"]

    in_maps = make_in_maps(x, w_qkv, w_proj, b_proj)
    res = run_bass_kernel_spmd(nc, in_maps, core_ids=list(range(B)))
    out = np.stack([res.results[b]["y"] for b in range(B)], axis=0)
    return out.astype(np.float32)
